# revision 4
# baseline (speedup 1.0000x reference)
"""Trainium2 Bass kernel for nn_Attention_59528246723073.

Reference (per batch b, channel c; x[b,c] is [S=256, T=64]):
    fs = tanh(x @ Wspect[c])            # [S]   (contract T)
    ft = tanh(x.T @ Wtemp[c])           # [T]   (contract S)
    a  = softmax_S(fs) * 100
    g  = softmax_T(ft)
    out[b,c,s,t] = x[b,c,s,t] * a[s] * g[t]

Distribution: data-parallel over batch B=32 -> 4 per core on 8 cores.

All tensors are marshaled to fp16 on the host (input cast + output upcast
are host-side numpy), so every DMA is a plain HWDGE transfer (sync/scalar
rings) and HBM traffic is halved vs f32.  Per-core layout: for each local
batch b, SBUF tile [128 part = channels, S*T free] fp16.  All big
elementwise ops are DVE fp16 tensor_tensor in the 2x_1p perf mode
(innermost step 1 on every operand), issued chunklessly (one FD=16384
instruction per pass) to minimize per-instruction overhead:
  - ft: tmp = x * wt_rep (pre-replicated Wtemp), then in-place fold chain
    over s down to FD=128 in fp16, finished in f32 for accuracy,
  - fs: tmp = x * Wspect broadcast over s (inner t contiguous), in-place
    folds over t to width 2, then one f32 tensor_reduce,
  - final: g-mul (inner-contiguous bcast) then a-mul via paired-duplicate
    a2[p, 2s+j] = a[p,s] so the broadcast keeps innermost step 1.
Softmax skips the max-subtraction: logits are tanh outputs in [-1, 1], so
exp never overflows and the exp's fused accum_out provides the sum.
The first batch is processed in graduated s-chunks so compute starts as
soon as the first piece of x has landed; the last batch's finals are split
into eighths so the tail out-DMA exposure is short.
"""

import numpy as np

import concourse.bass as bass
import concourse.tile as tile
from concourse import bacc, mybir
from concourse.bass_utils import run_bass_kernel_spmd

B, C, S, T = 32, 128, 256, 64
N_CORES = 8
B_LOC = B // N_CORES
F32 = mybir.dt.float32
F16 = mybir.dt.float16

_NC = None


def build_nc():
    nc = bacc.Bacc("TRN2", target_bir_lowering=False, debug=False)
    x = nc.dram_tensor("x", [B_LOC, C, S, T], F16, kind="ExternalInput")
    ws = nc.dram_tensor("wspect", [C, T], F16, kind="ExternalInput")
    wt = nc.dram_tensor("wtemp", [C, S], F16, kind="ExternalInput")
    out = nc.dram_tensor("out", [B_LOC, C, S, T], F16, kind="ExternalOutput")

    AF = mybir.ActivationFunctionType
    OP = mybir.AluOpType
    AX = mybir.AxisListType

    with tile.TileContext(nc) as tc:
        with (
            tc.tile_pool(name="consts", bufs=1) as cpool,
            tc.tile_pool(name="x2", bufs=2) as x2pool,
            tc.tile_pool(name="tmp", bufs=2) as tpool,
            tc.tile_pool(name="ocp", bufs=4) as ocpool,
            tc.tile_pool(name="small", bufs=2) as spool,
        ):
            # --- constants (fp16 straight from HBM via HWDGE) ---
            ws16 = cpool.tile([C, T], F16)
            nc.sync.dma_start(ws16[:], ws[:])
            wt16 = cpool.tile([C, S], F16)
            nc.sync.dma_start(wt16[:], wt[:])
            # wt_rep[c, s, t] = Wtemp[c, s] (fp16 contiguous), built in four
            # pieces on ScalarE so the first ft-mul isn't gated on one build.
            wt_rep = cpool.tile([C, S * T], F16)
            wt_rep3 = wt_rep.rearrange("p (s t) -> p s t", t=T)
            for q in range(4):
                sq = slice(q * S // 4, (q + 1) * S // 4)
                nc.scalar.activation(
                    wt_rep3[:, sq, :],
                    wt16[:, sq].unsqueeze(2).to_broadcast((C, S // 4, T)),
                    AF.Copy,
                )

            for b in range(B_LOC):
                X2 = x2pool.tile([C, S * T], F16, tag="X2")
                X23 = X2.rearrange("p (s t) -> p s t", t=T)
                fs = spool.tile([C, S], F32, tag="fs")
                ft = spool.tile([C, T], F32, tag="ft")
                fth = spool.tile([C, 2 * T], F16, tag="fth")
                ftf = spool.tile([C, 2 * T], F32, tag="ftf")

                # graduated chunks on the first batch: compute starts after
                # the first piece lands instead of after the full 2 MB.
                chunks = (32, 32, 64, 128) if b == 0 else (256,)
                s0 = 0
                for k, sc in enumerate(chunks):
                    sl = slice(s0, s0 + sc)
                    fsl = slice(s0 * T, (s0 + sc) * T)
                    with nc.named_scope("load"):
                        for q0 in range(s0, s0 + sc, 64):
                            sq = slice(q0, min(q0 + 64, s0 + sc))
                            nc.sync.dma_start(
                                X2[:, sq.start * T : sq.stop * T],
                                x[b, :, sq, :],
                            )

                    xc = X23[:, sl, :]
                    xcf = X2[:, fsl]
                    # ft-partial: fold x*wt over s (all-flat fp16 chain) down
                    # to FD=2T in fp16, finish in f32 for accuracy.  The first
                    # (largest) fold level runs on GpSimd to relieve the DVE.
                    with nc.named_scope("ft"):
                        tmp2 = tpool.tile([C, S * T], F16, tag="tmp")
                        nc.vector.tensor_tensor(
                            tmp2[:, fsl], xcf, wt_rep[:, fsl], op=OP.mult
                        )
                        w = sc * T // 2
                        first = True
                        while w >= 2 * T:
                            eng = nc.gpsimd if (first and sc == S) else nc.vector
                            eng.tensor_tensor(
                                tmp2[:, s0 * T : s0 * T + w],
                                tmp2[:, s0 * T : s0 * T + w],
                                tmp2[:, s0 * T + w : s0 * T + 2 * w],
                                op=OP.add,
                            )
                            first = False
                            w //= 2
                        # fp16 partial [C, 2T] -> f32, accumulate across chunks
                        if k == 0:
                            nc.vector.tensor_copy(
                                ftf[:], tmp2[:, s0 * T : s0 * T + 2 * T]
                            )
                        else:
                            nc.vector.tensor_copy(
                                fth[:], tmp2[:, s0 * T : s0 * T + 2 * T]
                            )
                            nc.vector.tensor_tensor(
                                ftf[:], ftf[:], fth[:], op=OP.add
                            )
                    # fs[:, sl] = sum_t xc * Wspect[:, None, :]
                    with nc.named_scope("fs"):
                        tmp = tpool.tile([C, S * T], F16, tag="tmp")
                        t3 = tmp.rearrange("p (s t) -> p s t", t=T)[:, sl, :]
                        nc.vector.tensor_tensor(
                            t3, xc, ws16.unsqueeze(1).to_broadcast((C, sc, T)),
                            op=OP.mult,
                        )
                        w = T // 2
                        first = True
                        while w >= 2:
                            eng = nc.gpsimd if (first and sc == S) else nc.vector
                            eng.tensor_tensor(
                                t3[:, :, 0:w], t3[:, :, 0:w],
                                t3[:, :, w : 2 * w], op=OP.add,
                            )
                            first = False
                            w //= 2
                        nc.vector.reduce_sum(fs[:, sl], t3[:, :, 0:2], axis=AX.X)
                    s0 += sc

                with nc.named_scope("softmax"):
                    # final f32 fold of ft partials: [C, 2T] -> [C, T]
                    nc.vector.tensor_tensor(
                        ft[:], ftf[:, 0:T], ftf[:, T : 2 * T], op=OP.add
                    )
                    # logits are tanh outputs in [-1,1]: no max-subtraction
                    # needed; exp's fused accum_out gives the softmax sum.
                    ssum = spool.tile([C, 1], F32, tag="ssum")
                    rec = spool.tile([C, 1], F32, tag="rec")
                    nc.scalar.activation(fs[:], fs[:], AF.Tanh)
                    nc.scalar.activation(
                        fs[:], fs[:], AF.Exp, accum_out=ssum[:, 0:1]
                    )
                    nc.vector.reciprocal(rec[:], ssum[:])
                    a2 = spool.tile([C, 2 * S], F16, tag="a2")
                    nc.vector.tensor_scalar(
                        out=a2.rearrange("p (s j) -> p s j", j=2),
                        in0=fs[:].unsqueeze(2).to_broadcast((C, S, 2)),
                        scalar1=rec[:, 0:1], scalar2=100.0,
                        op0=OP.mult, op1=OP.mult,
                    )

                    ssum2 = spool.tile([C, 1], F32, tag="ssum2")
                    rec2 = spool.tile([C, 1], F32, tag="rec2")
                    nc.scalar.activation(ft[:], ft[:], AF.Tanh)
                    nc.scalar.activation(
                        ft[:], ft[:], AF.Exp, accum_out=ssum2[:, 0:1]
                    )
                    nc.vector.reciprocal(rec2[:], ssum2[:])
                    g16 = spool.tile([C, T], F16, tag="g16")
                    nc.vector.tensor_scalar(
                        out=g16[:], in0=ft[:], scalar1=rec2[:, 0:1],
                        scalar2=None, op0=OP.mult,
                    )

                # final multiplies + store; eighths on the last batch so the
                # tail out-DMA exposure is short.
                nf = 8 if b == B_LOC - 1 else 4
                SQ = S // nf
                g_bcq = g16.unsqueeze(1).to_broadcast((C, SQ, T))
                for k in range(nf):
                    sl = slice(k * SQ, (k + 1) * SQ)
                    with nc.named_scope("final"):
                        oc = ocpool.tile([C, SQ * T], F16, tag="oc")
                        o3 = oc.rearrange("p (s t) -> p s t", t=T)
                        nc.vector.tensor_tensor(
                            o3, X23[:, sl, :], g_bcq, op=OP.mult
                        )
                        # a-mul on fp16 pairs: innermost step-1 j keeps 2x
                        oP = oc.rearrange(
                            "p (s pr j) -> p s pr j", pr=T // 2, j=2
                        )
                        aP = (
                            a2[:, 2 * k * SQ : 2 * (k + 1) * SQ]
                            .rearrange("p (s j) -> p s j", j=2)
                            .unsqueeze(2)
                            .to_broadcast((C, SQ, T // 2, 2))
                        )
                        nc.vector.tensor_tensor(oP, oP, aP, op=OP.mult)
                        nc.scalar.dma_start(out[b, :, sl, :], oc[:])

    nc.compile()
    return nc


def get_nc():
    global _NC
    if _NC is None:
        _NC = build_nc()
    return _NC


def shard_inputs(x, Wspect, Wtemp):
    ws = np.ascontiguousarray(Wspect.reshape(C, T).astype(np.float16))
    wt = np.ascontiguousarray(Wtemp.reshape(C, S).astype(np.float16))
    x = np.ascontiguousarray(x.astype(np.float16))
    return [
        {"x": x[i * B_LOC : (i + 1) * B_LOC], "wspect": ws, "wtemp": wt}
        for i in range(N_CORES)
    ]


def unshard(results):
    return np.concatenate([r["out"] for r in results], axis=0).astype(np.float32)


def kernel(x, Wspect, Wtemp):
    nc = get_nc()
    in_maps = shard_inputs(x, Wspect, Wtemp)
    res = run_bass_kernel_spmd(nc, in_maps, core_ids=list(range(N_CORES)))
    return unshard(res.results)


# revision 9
# speedup vs baseline: 1.2983x; 1.2983x over previous
"""Trainium2 Bass kernel for nn_Attention_59528246723073.

Reference (per batch b, channel c; x[b,c] is [S=256, T=64]):
    fs = tanh(x @ Wspect[c])            # [S]   (contract T)
    ft = tanh(x.T @ Wtemp[c])           # [T]   (contract S)
    a  = softmax_S(fs) * 100
    g  = softmax_T(ft)
    out[b,c,s,t] = x[b,c,s,t] * a[s] * g[t]

Distribution: data-parallel over batch B=32 -> 4 per core on 8 cores.

All tensors are marshaled to fp16 on the host (input cast + output upcast
are host-side numpy), so every DMA is a plain HWDGE transfer (sync/scalar
rings) and HBM traffic is halved vs f32.  Per-core layout: for each local
batch b, SBUF tile [128 part = channels, S*T free] fp16.  All big
elementwise ops are DVE fp16 tensor_tensor in the 2x_1p perf mode
(innermost step 1 on every operand), issued chunklessly (one FD=16384
instruction per pass) to minimize per-instruction overhead:
  - ft: tmp = x * wt_rep (pre-replicated Wtemp), then in-place fold chain
    over s down to FD=128 in fp16, finished in f32 for accuracy,
  - fs: tmp = x * Wspect broadcast over s (inner t contiguous), in-place
    folds over t to width 2, then one f32 tensor_reduce,
  - final: g-mul (inner-contiguous bcast) then a-mul via paired-duplicate
    a2[p, 2s+j] = a[p,s] so the broadcast keeps innermost step 1.
Softmax skips the max-subtraction: logits are tanh outputs in [-1, 1], so
exp never overflows and the exp's fused accum_out provides the sum.
The first batch is processed in graduated s-chunks so compute starts as
soon as the first piece of x has landed; the last batch's finals are split
into eighths so the tail out-DMA exposure is short.
"""

import numpy as np

import concourse.bass as bass
import concourse.tile as tile
from concourse import bacc, mybir
from concourse.bass_utils import run_bass_kernel_spmd

B, C, S, T = 32, 128, 256, 64
N_CORES = 8
B_LOC = B // N_CORES
F32 = mybir.dt.float32
F16 = mybir.dt.float16

_NC = None


def build_nc():
    nc = bacc.Bacc("TRN2", target_bir_lowering=False, debug=False)
    x = nc.dram_tensor("x", [B_LOC, C, S, T], F16, kind="ExternalInput")
    ws = nc.dram_tensor("wspect", [C, T], F16, kind="ExternalInput")
    wt = nc.dram_tensor("wtemp", [C, S], F16, kind="ExternalInput")
    out = nc.dram_tensor("out", [B_LOC, C, S, T], F16, kind="ExternalOutput")

    AF = mybir.ActivationFunctionType
    OP = mybir.AluOpType
    AX = mybir.AxisListType

    with tile.TileContext(nc) as tc:
        with (
            tc.tile_pool(name="consts", bufs=1) as cpool,
            tc.tile_pool(name="x2", bufs=2) as x2pool,
            tc.tile_pool(name="tmp", bufs=2) as tpool,
            tc.tile_pool(name="ocp", bufs=5) as ocpool,
            tc.tile_pool(name="small", bufs=2) as spool,
        ):
            # --- constants (fp16 straight from HBM via HWDGE) ---
            ws16 = cpool.tile([C, T], F16)
            nc.sync.dma_start(ws16[:], ws[:])
            wt16 = cpool.tile([C, S], F16)
            nc.sync.dma_start(wt16[:], wt[:])
            # wt_rep[c, s, t] = Wtemp[c, s] (fp16 contiguous), built in eight
            # pieces on ScalarE so the first ft-mul only gates on piece 0.
            wt_rep = cpool.tile([C, S * T], F16)
            wt_rep3 = wt_rep.rearrange("p (s t) -> p s t", t=T)
            for q in range(8):
                sq = slice(q * S // 8, (q + 1) * S // 8)
                nc.scalar.activation(
                    wt_rep3[:, sq, :],
                    wt16[:, sq].unsqueeze(2).to_broadcast((C, S // 8, T)),
                    AF.Copy,
                )

            for b in range(B_LOC):
                X2 = x2pool.tile([C, S * T], F16, tag="X2")
                X23 = X2.rearrange("p (s t) -> p s t", t=T)
                fs = spool.tile([C, S], F32, tag="fs")
                ft = spool.tile([C, T], F32, tag="ft")
                fth = spool.tile([C, 2 * T], F16, tag="fth")
                ftf = spool.tile([C, 2 * T], F32, tag="ftf")

                # graduated chunks on the first batch: compute starts after
                # the first piece lands instead of after the full 2 MB.
                chunks = (32, 32, 64, 128) if b == 0 else (256,)
                s0 = 0
                for k, sc in enumerate(chunks):
                    sl = slice(s0, s0 + sc)
                    fsl = slice(s0 * T, (s0 + sc) * T)
                    with nc.named_scope("load"):
                        for q0 in range(s0, s0 + sc, 64):
                            sq = slice(q0, min(q0 + 64, s0 + sc))
                            nc.sync.dma_start(
                                X2[:, sq.start * T : sq.stop * T],
                                x[b, :, sq, :],
                            )

                    xc = X23[:, sl, :]
                    xcf = X2[:, fsl]
                    # fs[:, sl] = sum_t xc * Wspect[:, None, :]
                    # (fs runs first: it only needs the tiny ws16 load, not
                    # the wt_rep build, so the ramp-in is shorter.)
                    with nc.named_scope("fs"):
                        tmp = tpool.tile([C, S * T], F16, tag="tmp")
                        t3 = tmp.rearrange("p (s t) -> p s t", t=T)[:, sl, :]
                        nc.vector.tensor_tensor(
                            t3, xc, ws16.unsqueeze(1).to_broadcast((C, sc, T)),
                            op=OP.mult,
                        )
                        w = T // 2
                        while w >= 2:
                            nc.vector.tensor_tensor(
                                t3[:, :, 0:w], t3[:, :, 0:w],
                                t3[:, :, w : 2 * w], op=OP.add,
                            )
                            w //= 2
                        nc.vector.reduce_sum(fs[:, sl], t3[:, :, 0:2], axis=AX.X)
                    # ft-partial: fold x*wt over s (all-flat fp16 chain) down
                    # to FD=2T in fp16, finish in f32 for accuracy.
                    with nc.named_scope("ft"):
                        tmp2 = tpool.tile([C, S * T], F16, tag="tmp")
                        nc.vector.tensor_tensor(
                            tmp2[:, fsl], xcf, wt_rep[:, fsl], op=OP.mult
                        )
                        w = sc * T // 2
                        while w >= 2 * T:
                            nc.vector.tensor_tensor(
                                tmp2[:, s0 * T : s0 * T + w],
                                tmp2[:, s0 * T : s0 * T + w],
                                tmp2[:, s0 * T + w : s0 * T + 2 * w],
                                op=OP.add,
                            )
                            w //= 2
                        # fp16 partial [C, 2T] -> f32, accumulate across chunks
                        if k == 0:
                            nc.vector.tensor_copy(
                                ftf[:], tmp2[:, s0 * T : s0 * T + 2 * T]
                            )
                        else:
                            nc.vector.tensor_copy(
                                fth[:], tmp2[:, s0 * T : s0 * T + 2 * T]
                            )
                            nc.vector.tensor_tensor(
                                ftf[:], ftf[:], fth[:], op=OP.add
                            )
                    s0 += sc

                with nc.named_scope("softmax"):
                    # final f32 fold of ft partials: [C, 2T] -> [C, T]
                    nc.vector.tensor_tensor(
                        ft[:], ftf[:, 0:T], ftf[:, T : 2 * T], op=OP.add
                    )
                    # logits are tanh outputs in [-1,1]: no max-subtraction
                    # needed; exp's fused accum_out gives the softmax sum.
                    ssum = spool.tile([C, 1], F32, tag="ssum")
                    rec = spool.tile([C, 1], F32, tag="rec")
                    nc.scalar.activation(fs[:], fs[:], AF.Tanh)
                    nc.scalar.activation(
                        fs[:], fs[:], AF.Exp, accum_out=ssum[:, 0:1]
                    )
                    nc.vector.reciprocal(rec[:], ssum[:])
                    a2 = spool.tile([C, 2 * S], F16, tag="a2")
                    nc.vector.tensor_scalar(
                        out=a2.rearrange("p (s j) -> p s j", j=2),
                        in0=fs[:].unsqueeze(2).to_broadcast((C, S, 2)),
                        scalar1=rec[:, 0:1], scalar2=100.0,
                        op0=OP.mult, op1=OP.mult,
                    )

                    ssum2 = spool.tile([C, 1], F32, tag="ssum2")
                    rec2 = spool.tile([C, 1], F32, tag="rec2")
                    nc.scalar.activation(ft[:], ft[:], AF.Tanh)
                    nc.scalar.activation(
                        ft[:], ft[:], AF.Exp, accum_out=ssum2[:, 0:1]
                    )
                    nc.vector.reciprocal(rec2[:], ssum2[:])
                    g16 = spool.tile([C, T], F16, tag="g16")
                    nc.vector.tensor_scalar(
                        out=g16[:], in0=ft[:], scalar1=rec2[:, 0:1],
                        scalar2=None, op0=OP.mult,
                    )

                # final multiplies + store; eighths on the last batch so the
                # tail out-DMA exposure is short.
                nf = 8 if b == B_LOC - 1 else 4
                SQ = S // nf
                g_bcq = g16.unsqueeze(1).to_broadcast((C, SQ, T))
                for k in range(nf):
                    sl = slice(k * SQ, (k + 1) * SQ)
                    with nc.named_scope("final"):
                        oc = ocpool.tile([C, SQ * T], F16, tag="oc")
                        o3 = oc.rearrange("p (s t) -> p s t", t=T)
                        nc.vector.tensor_tensor(
                            o3, X23[:, sl, :], g_bcq, op=OP.mult
                        )
                        # a-mul on fp16 pairs: innermost step-1 j keeps 2x
                        oP = oc.rearrange(
                            "p (s pr j) -> p s pr j", pr=T // 2, j=2
                        )
                        aP = (
                            a2[:, 2 * k * SQ : 2 * (k + 1) * SQ]
                            .rearrange("p (s j) -> p s j", j=2)
                            .unsqueeze(2)
                            .to_broadcast((C, SQ, T // 2, 2))
                        )
                        nc.vector.tensor_tensor(oP, oP, aP, op=OP.mult)
                        # alternate the two HWDGE rings so stores never queue
                        # behind each other on one ring
                        eng = nc.scalar if k % 2 == 0 else nc.sync
                        eng.dma_start(out[b, :, sl, :], oc[:])

    nc.compile()
    return nc


def get_nc():
    global _NC
    if _NC is None:
        _NC = build_nc()
    return _NC


def shard_inputs(x, Wspect, Wtemp):
    ws = np.ascontiguousarray(Wspect.reshape(C, T).astype(np.float16))
    wt = np.ascontiguousarray(Wtemp.reshape(C, S).astype(np.float16))
    x = np.ascontiguousarray(x.astype(np.float16))
    return [
        {"x": x[i * B_LOC : (i + 1) * B_LOC], "wspect": ws, "wtemp": wt}
        for i in range(N_CORES)
    ]


def unshard(results):
    return np.concatenate([r["out"] for r in results], axis=0).astype(np.float32)


def kernel(x, Wspect, Wtemp):
    nc = get_nc()
    in_maps = shard_inputs(x, Wspect, Wtemp)
    res = run_bass_kernel_spmd(nc, in_maps, core_ids=list(range(N_CORES)))
    return unshard(res.results)
